# revision 1
# baseline (speedup 1.0000x reference)
"""Trainium2 Bass kernel for nn_Criterion_37984690765901.

Loss =  L_t + lam_e * Loss_e + lam_od * (L_zt + L_zs)
  L_t    = mean_r( lse(y_zt_r) - y_zt[r, target_r] )            (cross entropy)
  Loss_e = mean_r( lse(s_r) - (sum_j e^{s_rj} s_rj)/sum_j e^{s_rj} )   (entropy)
  L_zt/L_zs = mean_r( rowdot_r/s_r - ln s_r + ln ps_r )          (KLD batchmean)
     with enc = mean + exp(0.5*log_std)*eps,  e = exp(enc), s = sum_d e,
     pe = exp(prior), ps = sum_d pe, rowdot = sum_d e*(enc - prior).
     (prior_s = 1 + eps_prior_s, but KLD is shift-invariant in the prior
      logits, so eps_prior_s is used directly.)

Sharding: pure data parallel over the batch axis, 8192 rows per core.
Each [8192, D] shard is viewed as [128, 8192] (partition p holds rows
64p..64p+63 contiguously); all per-row reductions are free-axis segmented
reduces, and the batch reduction is finished on the host in float64.

Scheduling note: walrus allows a single sync-wait command per DVE
instruction, so the per-chunk op order is arranged such that every
instruction needs at most one unobserved cross-engine semaphore (the
PS-reduce observes ACT first; y_zt and its one-hot ride one DMA).

Device per-core outputs: out[128, 256] f32 =
  [:, 0:64]    per-row KL contribution, t branch
  [:, 64:128]  per-row KL contribution, s branch
  [:, 128:192] per-row (lse_y - y_pick)
  [:, 192:256] per-row entropy of softmax(s_zt)
"""

import os
import numpy as np

NCORES = 8
B, D, C, S = 65536, 128, 10, 2
LAMBDA_E, LAMBDA_OD = 0.1, 0.036
GAMMA_E, GAMMA_OD = 2.0, 2.0
STEP_SIZE = 1000.0

RPC = B // NCORES            # rows per core = 8192
P = 128                      # SBUF partitions
FREE = RPC * D // P          # 8192 free elems per partition per big tensor
CHUNK = 2048                 # free elems per chunk
G = CHUNK // D               # 16 row-groups per chunk
NCH = FREE // CHUNK          # 4 chunks per tensor
NCOL = FREE // D             # 64 rows per partition (stat columns)
YF = RPC * C // P            # 640
SF = RPC * S // P            # 128

# packed per-branch DRAM tensors: [P, NCH, 4*CHUNK] with chunk layout
# [log_std | prior | eps | mean]; DMA pair A = ACT inputs (std, pe),
# DMA pair B = DVE inputs (eps, mean)
BRANCHES = ["bt", "bs"]

# A/B knob: run the se = std*eps multiply on GPSIMD instead of DVE
SE_ON_GPSIMD = False

_CACHED_NC = None
LAST_EXEC_NS = None


def _build_nc():
    import concourse.bass as bass
    import concourse.tile as tile
    from concourse import mybir
    from contextlib import ExitStack

    f32 = mybir.dt.float32
    Exp = mybir.ActivationFunctionType.Exp
    Ln = mybir.ActivationFunctionType.Ln
    add = mybir.AluOpType.add
    sub = mybir.AluOpType.subtract
    mult = mybir.AluOpType.mult
    X = mybir.AxisListType.X

    nc = bass.Bass("TRN2", debug=False)

    ins = {}
    for bn in BRANCHES:
        ins[bn] = nc.dram_tensor(
            bn, [P, NCH, 4 * CHUNK], f32, kind="ExternalInput"
        ).ap()
    ins["yoh"] = nc.dram_tensor("yoh", [P, 2 * YF], f32, kind="ExternalInput").ap()
    ins["sz"] = nc.dram_tensor("sz", [P, SF], f32, kind="ExternalInput").ap()
    out_d = nc.dram_tensor("out", [P, 4 * NCOL], f32, kind="ExternalOutput").ap()

    se_eng = nc.gpsimd if SE_ON_GPSIMD else nc.vector

    with tile.TileContext(nc) as tc, ExitStack() as ctx:
        io = ctx.enter_context(tc.tile_pool(name="io", bufs=5))
        pep = ctx.enter_context(tc.tile_pool(name="pep", bufs=1))
        st = ctx.enter_context(tc.tile_pool(name="st", bufs=1))

        out_sb = st.tile([P, 4 * NCOL], f32, tag="out")

        # SRD[:, 0, :] = per-row sum(e);  SRD[:, 1, :] = per-row sum(e*d)
        SRD_ts = [
            st.tile([P, 2, NCOL], f32, tag=f"SRD{b}", name=f"SRD{b}")
            for b in range(2)
        ]
        PS_ts = [
            st.tile([P, NCOL], f32, tag=f"PS{b}", name=f"PS{b}")
            for b in range(2)
        ]

        # Software-pipelined emission over interleaved branches:
        #   S0(s): DMA chunk s
        #   S1(s): ACT std/pe; DVE ps-red, se, enc; ACT e
        #   S2(s): DVE d, ed, combined [e|ed] reduce
        # The one-step lag between S1 and S2 gives the DVE stream
        # independent work while ACT computes exp(enc).
        NSTEPS = 2 * NCH
        state = {}

        def stage0(s):
            b, c = s % 2, s // 2
            t = io.tile([P, 4 * CHUNK], f32, tag="pk", name=f"pk{s}")
            # per-slice DMAs in dependency order (log_std and prior first,
            # so ACT's std/pe start after ~1MB instead of ~4MB)
            for k in range(4):
                nc.sync.dma_start(
                    t[:, bass.ts(k, CHUNK)],
                    ins[BRANCHES[b]][:, c, bass.ts(k, CHUNK)],
                )
            state[s] = t

        # slice lifetimes: 0: log_std -> std -> e;  1: prior -> d -> ed;
        #                  2: eps -> se;             3: mean -> enc
        def stage1a(s):
            # ACT std/pe + the PS reduce; emitted one step ahead of the
            # exp(enc) so the in-order ACT stream never delays the next
            # chunk's std/pe behind a DVE-dependent exp.
            b, c = s % 2, s // 2
            t = state[s]
            l_ap = t[:, 0 * CHUNK:1 * CHUNK]
            p_ap = t[:, 1 * CHUNK:2 * CHUNK]
            nc.scalar.activation(l_ap, l_ap, Exp, scale=0.5)
            pe_t = pep.tile([P, CHUNK], f32, tag="pe", name=f"pe{s}")
            nc.scalar.activation(pe_t[:], p_ap, Exp)
            # DVE observes ACT here (covers std + pe ticks)
            nc.vector.tensor_reduce(
                PS_ts[b][:, bass.ts(c, G)],
                pe_t[:].rearrange("p (g d) -> p g d", d=D), X, add
            )

        def stage1b(s):
            t = state[s]
            l_ap = t[:, 0 * CHUNK:1 * CHUNK]
            e_ap = t[:, 2 * CHUNK:3 * CHUNK]
            m_ap = t[:, 3 * CHUNK:4 * CHUNK]
            # se = std * eps           (into eps slice)
            se_eng.tensor_tensor(e_ap, l_ap, e_ap, mult)
            # enc = se + mean          (into mean slice)
            nc.vector.tensor_tensor(m_ap, e_ap, m_ap, add)
            # e = exp(enc)             (ACT, into dead std slice)
            nc.scalar.activation(l_ap, m_ap, Exp)

        def stage2(s):
            b, c = s % 2, s // 2
            t = state.pop(s)
            l_ap = t[:, 0 * CHUNK:1 * CHUNK]   # e
            p_ap = t[:, 1 * CHUNK:2 * CHUNK]   # prior -> d -> ed
            m_ap = t[:, 3 * CHUNK:4 * CHUNK]   # enc
            # d = enc - prior          (into prior slice)
            nc.vector.tensor_tensor(p_ap, m_ap, p_ap, sub)
            # ed = e * d               (in place over d, next to e)
            nc.vector.tensor_tensor(p_ap, l_ap, p_ap, mult)
            # combined segmented reduce over adjacent [e | ed] slices:
            # [P, 2, G, D] -> [P, 2, G]
            nc.vector.tensor_reduce(
                SRD_ts[b][:, :, bass.ts(c, G)],
                t[:, 0:2 * CHUNK].rearrange("p (k g d) -> p k g d", k=2, d=D),
                X, add,
            )

        # --- small blocks first: their DMA + compute fill the pipeline
        # warm-up while the first big chunks stream in ---
        # cross entropy on y_zt: per-row lse - picked
        yoh_t = st.tile([P, 2 * YF], f32, tag="yoh")
        nc.sync.dma_start(yoh_t[:], ins["yoh"][:])
        y_ap = yoh_t[:, 0:YF]
        oh_ap = yoh_t[:, YF:2 * YF]
        ey_t = st.tile([P, YF], f32, tag="ey")
        nc.scalar.activation(ey_t[:], y_ap, Exp)
        sy_t = st.tile([P, NCOL], f32, tag="sy")
        nc.vector.tensor_reduce(
            sy_t[:], ey_t[:].rearrange("p (g c) -> p g c", c=C), X, add
        )
        lse_t = st.tile([P, NCOL], f32, tag="lse")
        nc.scalar.activation(lse_t[:], sy_t[:], Ln)
        ym_t = st.tile([P, YF], f32, tag="ym")
        nc.vector.tensor_tensor(ym_t[:], y_ap, oh_ap, mult)
        pick_t = st.tile([P, NCOL], f32, tag="pick")
        nc.vector.tensor_reduce(
            pick_t[:], ym_t[:].rearrange("p (g c) -> p g c", c=C), X, add
        )
        nc.vector.tensor_tensor(
            out_sb[:, bass.ts(2, NCOL)], lse_t[:], pick_t[:], sub
        )
        nc.sync.dma_start(out_d[:, bass.ts(2, NCOL)], out_sb[:, bass.ts(2, NCOL)])

        # entropy of softmax(s_zt): per-row lse - (sum e*x)/s
        sz_t = st.tile([P, SF], f32, tag="sz")
        nc.sync.dma_start(sz_t[:], ins["sz"][:])
        esz_t = st.tile([P, SF], f32, tag="esz")
        nc.scalar.activation(esz_t[:], sz_t[:], Exp)
        ssum_t = st.tile([P, NCOL], f32, tag="ssum")
        nc.vector.tensor_reduce(
            ssum_t[:], esz_t[:].rearrange("p (g c) -> p g c", c=S), X, add
        )
        exs_t = st.tile([P, SF], f32, tag="exs")
        nc.vector.tensor_tensor(exs_t[:], esz_t[:], sz_t[:], mult)
        dsum_t = st.tile([P, NCOL], f32, tag="dsum")
        nc.vector.tensor_reduce(
            dsum_t[:], exs_t[:].rearrange("p (g c) -> p g c", c=S), X, add
        )
        rss_t = st.tile([P, NCOL], f32, tag="rss")
        nc.vector.reciprocal(rss_t[:], ssum_t[:])
        t2_t = st.tile([P, NCOL], f32, tag="t2")
        nc.vector.tensor_tensor(t2_t[:], dsum_t[:], rss_t[:], mult)
        lss_t = st.tile([P, NCOL], f32, tag="lss")
        nc.scalar.activation(lss_t[:], ssum_t[:], Ln)
        nc.vector.tensor_tensor(
            out_sb[:, bass.ts(3, NCOL)], lss_t[:], t2_t[:], sub
        )
        nc.sync.dma_start(out_d[:, bass.ts(3, NCOL)], out_sb[:, bass.ts(3, NCOL)])

        for i in range(NSTEPS + 3):
            if i < NSTEPS:
                stage0(i)
            if 1 <= i and i - 1 < NSTEPS:
                stage1a(i - 1)
            if 2 <= i and i - 2 < NSTEPS:
                stage1b(i - 2)
            if 3 <= i and i - 3 < NSTEPS:
                stage2(i - 3)

        # tails: kl_row = RD/S - ln S + ln PS  (once per branch)
        for b in range(2):
            SRD_t, PS_t = SRD_ts[b], PS_ts[b]
            S_ap = SRD_t[:, 0, :]
            RD_ap = SRD_t[:, 1, :]
            rs_t = st.tile([P, NCOL], f32, tag=f"rs{b}")
            nc.vector.reciprocal(rs_t[:], S_ap)
            term_t = st.tile([P, NCOL], f32, tag=f"term{b}")
            nc.vector.tensor_tensor(term_t[:], RD_ap, rs_t[:], mult)
            lnS_t = st.tile([P, NCOL], f32, tag=f"lnS{b}")
            nc.scalar.activation(lnS_t[:], S_ap, Ln)
            lnPS_t = st.tile([P, NCOL], f32, tag=f"lnPS{b}")
            nc.scalar.activation(lnPS_t[:], PS_t[:], Ln)
            tmp_t = st.tile([P, NCOL], f32, tag=f"tmp{b}")
            nc.vector.tensor_tensor(tmp_t[:], term_t[:], lnS_t[:], sub)
            nc.vector.tensor_tensor(
                out_sb[:, bass.ts(b, NCOL)], tmp_t[:], lnPS_t[:], add
            )
            nc.sync.dma_start(
                out_d[:, bass.ts(b, NCOL)], out_sb[:, bass.ts(b, NCOL)]
            )

    return nc


def _split_multi_waits(nc):
    """walrus's codegen allows a single embedded sync-wait per compute
    instruction; Tile sometimes emits two (e.g. ACT + DMA deps on one TT).
    Hoist all-but-one wait into standalone EventSemaphore instructions
    placed immediately before, on the same engine. Applied at BIR-JSON
    serialization time so CoreSim (which handles multi-wait fine) is
    untouched."""
    import json

    orig = nc.to_json_bytes

    def patched():
        bj = json.loads(orig())
        for fn in bj["functions"]:
            for blk in fn["blocks"]:
                new = []
                for inst in blk["instructions"]:
                    si = inst.get("sync_info") or {}
                    waits = si.get("on_wait") or []
                    if len(waits) > 1 and inst.get("opcode") != "EventSemaphore":
                        for i, w in enumerate(waits[:-1]):
                            new.append({
                                "debug": inst.get("debug"),
                                "engine": inst["engine"],
                                "ins": [],
                                "name": f"{inst['name']}-sw{i}",
                                "opcode": "EventSemaphore",
                                "outs": [],
                                "sync_info": {"on_update": [], "on_wait": [w]},
                            })
                        si["on_wait"] = [waits[-1]]
                    new.append(inst)
                blk["instructions"] = new
        return json.dumps(bj).encode()

    nc.to_json_bytes = patched
    return nc


def get_nc():
    global _CACHED_NC
    if _CACHED_NC is None:
        _CACHED_NC = _split_multi_waits(_build_nc())
    return _CACHED_NC


def make_in_maps(inputs):
    """Shard the full inputs into per-core in_maps for run_bass_kernel_spmd."""
    f32 = np.float32
    arr = {k: np.asarray(v) for k, v in inputs.items()}
    target = np.asarray(arr["target"]).astype(np.int64).reshape(B)
    onehot = np.zeros((B, C), dtype=f32)
    onehot[np.arange(B), target] = 1.0

    branch_srcs = {
        "bt": ("log_std_t", "eps_prior_t", "eps_t", "mean_t"),
        "bs": ("log_std_s", "eps_prior_s", "eps_s", "mean_s"),
    }
    in_maps = []
    for cidx in range(NCORES):
        sl = slice(cidx * RPC, (cidx + 1) * RPC)
        m = {}
        for bn, srcs in branch_srcs.items():
            # [P, NCH, 4, CHUNK]: chunk c holds [log_std | prior | eps | mean]
            pk = np.stack(
                [
                    np.ascontiguousarray(arr[s][sl], dtype=f32).reshape(
                        P, NCH, CHUNK)
                    for s in srcs
                ],
                axis=2,
            )
            m[bn] = pk.reshape(P, NCH, 4 * CHUNK)
        yoh = np.empty((P, 2 * YF), dtype=f32)
        yoh[:, :YF] = np.ascontiguousarray(arr["y_zt"][sl], dtype=f32).reshape(P, YF)
        yoh[:, YF:] = np.ascontiguousarray(onehot[sl]).reshape(P, YF)
        m["yoh"] = yoh
        m["sz"] = np.ascontiguousarray(arr["s_zt"][sl], dtype=f32).reshape(P, SF)
        in_maps.append(m)
    return in_maps


def combine(outs, current_step):
    """Host-side unshard: f64 reduce of per-row partials -> final f32 scalar."""
    tot = np.zeros(4, dtype=np.float64)
    for o in outs:
        o = o.reshape(P, 4, NCOL)
        tot += o.sum(axis=(0, 2), dtype=np.float64)
    L_zt, L_zs, L_t, Loss_e = tot / B
    frac = float(current_step) / STEP_SIZE
    lam_e = LAMBDA_E * GAMMA_E ** frac
    lam_od = LAMBDA_OD * GAMMA_OD ** frac
    val = L_t + lam_e * Loss_e + lam_od * (L_zt + L_zs)
    return np.array(val, dtype=np.float32)


def _install_ntff_hook():
    """Best-effort: register the axon NTFF profiling hook that the agent
    image's antenv package is missing, so trace=True yields exec_time_ns."""
    try:
        import sys, types
        import antenv
        if "antenv.axon_hooks" in sys.modules:
            return True
        sys.path.insert(0, "/root/.axon_site/trn_agent_boot")
        import trn_boot
        mod = types.ModuleType("antenv.axon_hooks")
        _h = {}
        mod.set_axon_ntff_profile_hook = lambda h: _h.__setitem__("h", h)
        mod.get_axon_ntff_profile_hook = lambda: _h.get("h")
        sys.modules["antenv.axon_hooks"] = mod
        antenv.axon_hooks = mod
        mod.set_axon_ntff_profile_hook(
            trn_boot._ntff_profile_via_ctypes("/opt/axon/libaxon_pjrt.so")
        )
        import concourse.bass_utils as bu
        bu.upload_artifacts = lambda tmpdir: str(tmpdir)
        return True
    except Exception:
        return False


def kernel(**inputs):
    global LAST_EXEC_NS
    from concourse.bass_utils import run_bass_kernel_spmd

    trace = os.environ.get("BASS_KERNEL_TRACE", "0") == "1"
    if trace:
        trace = _install_ntff_hook()

    nc = get_nc()
    in_maps = make_in_maps(inputs)
    res = run_bass_kernel_spmd(
        nc, in_maps, list(range(NCORES)), trace=trace
    )
    LAST_EXEC_NS = res.exec_time_ns
    outs = [r["out"] for r in res.results]
    cs = inputs.get("current_step", 500)
    return combine(outs, int(np.asarray(cs)))



# revision 3
# speedup vs baseline: 2.0078x; 2.0078x over previous
"""Trainium2 Bass kernel for nn_Criterion_37984690765901.

Loss =  L_t + lam_e * Loss_e + lam_od * (L_zt + L_zs)
  L_t    = mean_r( lse(y_zt_r) - y_zt[r, target_r] )            (cross entropy)
  Loss_e = mean_r( lse(s_r) - (sum_j e^{s_rj} s_rj)/sum_j e^{s_rj} )   (entropy)
  L_zt/L_zs = mean_r( rowdot_r/S_r - ln S_r + ln PS_r )          (KLD batchmean)
     with enc = mean + exp(0.5*log_std)*eps,  e = exp(enc), S = sum_d e,
     pe = exp(prior), PS = sum_d pe, rowdot = sum_d e*(enc - prior).
     (prior_s = 1 + eps_prior_s, but KLD is shift-invariant in the prior
      logits, so eps_prior_s is used directly.)

v2 design (vs the f32 rows-on-partitions baseline at 151.6us):
  * All big tensors are shipped as bf16 (halves HBM traffic; host sim of the
    full quantization chain gives rel err 3.9e-6 vs the 2e-2 gate).
  * Layout flip: D=128 lives on SBUF partitions, rows on the free axis.
    Elementwise ops are layout-agnostic, DVE tensor_tensor ops run at
    2x (16-bit packed), and all three per-row segmented reductions
    (S = sum_d e, RD = sum_d e*d, PS = sum_d pe) move to the otherwise-idle
    TensorEngine as data-as-stationary x ones[128,1] matmuls, each landing
    a [128 rows, 1] PSUM column. DVE drops from ~131us busy to ~4 TT/chunk.
  * log_std is pre-halved on the host so [0.5*ls | prior] exp in 1 ACT op.

Sharding: pure data parallel over the batch axis, 8192 rows per core.
Per-core per-branch DRAM: [128, NCH, 4*R] bf16, chunk c holds
[0.5*log_std | prior | eps | mean], each slice [128 D, R rows].

Device per-core outputs: out[128, 256] f32 =
  [:, 0:64]    per-row KL contribution, t branch   (row = col*128 + p)
  [:, 64:128]  per-row KL contribution, s branch
  [:, 128:192] per-row (lse_y - y_pick)            (row = 64*p + j)
  [:, 192:256] per-row entropy of softmax(s_zt)
Host combine just sums everything in f64, so orderings don't matter.
"""

import os
import numpy as np

NCORES = 8
B, D, C, S = 65536, 128, 10, 2
LAMBDA_E, LAMBDA_OD = 0.1, 0.036
GAMMA_E, GAMMA_OD = 2.0, 2.0
STEP_SIZE = 1000.0

RPC = B // NCORES            # rows per core = 8192
P = 128                      # SBUF partitions = D
R = 2048                     # rows per chunk
NCH = RPC // R               # 4 chunks per branch
NBLK = R // 128              # 16 row-blocks (matmuls) per chunk per stat
NCOLS = RPC // 128           # 64 stat columns per branch
NCOL = 64                    # per-row stat cols in the small-block layout
YF = RPC * C // P            # 640
SF = RPC * S // P            # 128

BRANCHES = ["bt", "bs"]

_CACHED_NC = None
LAST_EXEC_NS = None


def _build_nc():
    import concourse.bass as bass
    import concourse.tile as tile
    from concourse import mybir
    from contextlib import ExitStack

    f32 = mybir.dt.float32
    bf16 = mybir.dt.bfloat16
    Exp = mybir.ActivationFunctionType.Exp
    Ln = mybir.ActivationFunctionType.Ln
    add = mybir.AluOpType.add
    sub = mybir.AluOpType.subtract
    mult = mybir.AluOpType.mult
    X = mybir.AxisListType.X

    nc = bass.Bass("TRN2", debug=False)

    ins = {}
    for bn in BRANCHES:
        ins[bn] = nc.dram_tensor(
            bn, [P, NCH, 4 * R], bf16, kind="ExternalInput"
        ).ap()
    ins["yoh"] = nc.dram_tensor("yoh", [P, 2 * YF], bf16, kind="ExternalInput").ap()
    ins["sz"] = nc.dram_tensor("sz", [P, SF], bf16, kind="ExternalInput").ap()
    out_d = nc.dram_tensor("out", [P, 4 * NCOL], f32, kind="ExternalOutput").ap()

    with tile.TileContext(nc) as tc, ExitStack() as ctx:
        io = ctx.enter_context(tc.tile_pool(name="io", bufs=4))
        pep = ctx.enter_context(tc.tile_pool(name="pep", bufs=3))
        eep = ctx.enter_context(tc.tile_pool(name="eep", bufs=3))
        st = ctx.enter_context(tc.tile_pool(name="st", bufs=1))
        ps = ctx.enter_context(tc.psum_pool(name="ps", bufs=1))

        out_sb = st.tile([P, 4 * NCOL], f32, tag="out")

        # PSUM stat tiles: column col = chunk*NBLK + blk holds rows
        # col*128 .. col*128+127 of this core's shard.
        PS_S = [ps.tile([P, NCOLS], f32, tag=f"S{b}", name=f"S{b}")
                for b in range(2)]
        PS_RD = [ps.tile([P, NCOLS], f32, tag=f"RD{b}", name=f"RD{b}")
                 for b in range(2)]
        PS_PS = [ps.tile([P, NCOLS], f32, tag=f"PS{b}", name=f"PS{b}")
                 for b in range(2)]

        ones_t = st.tile([P, 1], bf16, tag="ones")
        nc.vector.memset(ones_t[:], 1.0)

        NSTEPS = 2 * NCH
        state = {}

        def stage0(s):
            # DMA chunk s: [0.5*ls | prior] first (feeds ACT soonest)
            b, c = s % 2, s // 2
            t = io.tile([P, 4 * R], bf16, tag="pk", name=f"pk{s}")
            nc.sync.dma_start(t[:, 0:2 * R], ins[BRANCHES[b]][:, c, 0:2 * R])
            nc.sync.dma_start(t[:, 2 * R:4 * R], ins[BRANCHES[b]][:, c, 2 * R:4 * R])
            state[s] = t

        def stage1(s):
            # ACT: [std | pe] = exp([0.5*ls | prior]) in one instruction
            t = state[s]
            sp = pep.tile([P, 2 * R], bf16, tag="sp", name=f"sp{s}")
            nc.scalar.activation(sp[:], t[:, 0:2 * R], Exp)
            state[(s, "sp")] = sp

        def stage2(s):
            # DVE se/enc (2x bf16), ACT e, DVE d/ed; all stay in SBUF
            t = state[s]
            sp = state[(s, "sp")]
            # se = std * eps            (into eps slot)
            nc.vector.tensor_tensor(
                t[:, 2 * R:3 * R], sp[:, 0:R], t[:, 2 * R:3 * R], mult)
            # enc = se + mean           (into mean slot)
            nc.vector.tensor_tensor(
                t[:, 3 * R:4 * R], t[:, 2 * R:3 * R], t[:, 3 * R:4 * R], add)
            ee = eep.tile([P, 2 * R], bf16, tag="ee", name=f"ee{s}")
            # e = exp(enc)
            nc.scalar.activation(ee[:, 0:R], t[:, 3 * R:4 * R], Exp)
            # d = enc - prior           (into dead ls slot; overlaps ACT e)
            nc.vector.tensor_tensor(
                t[:, 0:R], t[:, 3 * R:4 * R], t[:, R:2 * R], sub)
            # ed = e * d
            nc.vector.tensor_tensor(ee[:, R:2 * R], ee[:, 0:R], t[:, 0:R], mult)
            state[(s, "ee")] = ee

        def stage3(s):
            # PE: 48 matmuls; each reduces a [128 D, 128 rows] block against
            # ones -> one PSUM column of per-row sums. Emitted in readiness
            # order (pe, then e, then ed).
            b, c = s % 2, s // 2
            sp = state.pop((s, "sp"))
            ee = state.pop((s, "ee"))
            state.pop(s)
            for j in range(NBLK):
                col = c * NBLK + j
                nc.tensor.matmul(
                    PS_PS[b][:, col:col + 1],
                    sp[:, R + 128 * j:R + 128 * (j + 1)], ones_t[:],
                    start=True, stop=True)
            for j in range(NBLK):
                col = c * NBLK + j
                nc.tensor.matmul(
                    PS_S[b][:, col:col + 1],
                    ee[:, 128 * j:128 * (j + 1)], ones_t[:],
                    start=True, stop=True)
            for j in range(NBLK):
                col = c * NBLK + j
                nc.tensor.matmul(
                    PS_RD[b][:, col:col + 1],
                    ee[:, R + 128 * j:R + 128 * (j + 1)], ones_t[:],
                    start=True, stop=True)

        # --- small blocks first: their DMA + compute fill the pipeline
        # warm-up while the first big chunks stream in ---
        # cross entropy on y_zt: per-row lse - picked
        yoh_t = st.tile([P, 2 * YF], bf16, tag="yoh")
        nc.sync.dma_start(yoh_t[:], ins["yoh"][:])
        y_ap = yoh_t[:, 0:YF]
        oh_ap = yoh_t[:, YF:2 * YF]
        ey_t = st.tile([P, YF], bf16, tag="ey")
        nc.scalar.activation(ey_t[:], y_ap, Exp)
        sy_t = st.tile([P, NCOL], f32, tag="sy")
        nc.vector.tensor_reduce(
            sy_t[:], ey_t[:].rearrange("p (g c) -> p g c", c=C), X, add
        )
        lse_t = st.tile([P, NCOL], f32, tag="lse")
        nc.scalar.activation(lse_t[:], sy_t[:], Ln)
        ym_t = st.tile([P, YF], bf16, tag="ym")
        nc.vector.tensor_tensor(ym_t[:], y_ap, oh_ap, mult)
        pick_t = st.tile([P, NCOL], f32, tag="pick")
        nc.vector.tensor_reduce(
            pick_t[:], ym_t[:].rearrange("p (g c) -> p g c", c=C), X, add
        )
        nc.vector.tensor_tensor(
            out_sb[:, 2 * NCOL:3 * NCOL], lse_t[:], pick_t[:], sub
        )
        nc.sync.dma_start(out_d[:, 2 * NCOL:3 * NCOL], out_sb[:, 2 * NCOL:3 * NCOL])

        # entropy of softmax(s_zt): per-row lse - (sum e*x)/s
        sz_t = st.tile([P, SF], bf16, tag="sz")
        nc.sync.dma_start(sz_t[:], ins["sz"][:])
        esz_t = st.tile([P, SF], bf16, tag="esz")
        nc.scalar.activation(esz_t[:], sz_t[:], Exp)
        ssum_t = st.tile([P, NCOL], f32, tag="ssum")
        nc.vector.tensor_reduce(
            ssum_t[:], esz_t[:].rearrange("p (g c) -> p g c", c=S), X, add
        )
        exs_t = st.tile([P, SF], bf16, tag="exs")
        nc.vector.tensor_tensor(exs_t[:], esz_t[:], sz_t[:], mult)
        dsum_t = st.tile([P, NCOL], f32, tag="dsum")
        nc.vector.tensor_reduce(
            dsum_t[:], exs_t[:].rearrange("p (g c) -> p g c", c=S), X, add
        )
        rss_t = st.tile([P, NCOL], f32, tag="rss")
        nc.vector.reciprocal(rss_t[:], ssum_t[:])
        t2_t = st.tile([P, NCOL], f32, tag="t2")
        nc.vector.tensor_tensor(t2_t[:], dsum_t[:], rss_t[:], mult)
        lss_t = st.tile([P, NCOL], f32, tag="lss")
        nc.scalar.activation(lss_t[:], ssum_t[:], Ln)
        nc.vector.tensor_tensor(
            out_sb[:, 3 * NCOL:4 * NCOL], lss_t[:], t2_t[:], sub
        )
        nc.sync.dma_start(out_d[:, 3 * NCOL:4 * NCOL], out_sb[:, 3 * NCOL:4 * NCOL])

        # --- big-branch software pipeline ---
        for i in range(NSTEPS + 2):
            if i < NSTEPS:
                stage0(i)
            if 1 <= i and i - 1 < NSTEPS:
                stage1(i - 1)
            if 2 <= i and i - 2 < NSTEPS:
                stage2(i - 2)
                stage3(i - 2)

        # tails: kl_row = RD/S - ln S + ln PS  (once per branch)
        for b in range(2):
            rs_t = st.tile([P, NCOLS], f32, tag=f"rs{b}")
            nc.vector.reciprocal(rs_t[:], PS_S[b][:])
            term_t = st.tile([P, NCOLS], f32, tag=f"term{b}")
            nc.vector.tensor_tensor(term_t[:], PS_RD[b][:], rs_t[:], mult)
            lnS_t = st.tile([P, NCOLS], f32, tag=f"lnS{b}")
            nc.scalar.activation(lnS_t[:], PS_S[b][:], Ln)
            lnPS_t = st.tile([P, NCOLS], f32, tag=f"lnPS{b}")
            nc.scalar.activation(lnPS_t[:], PS_PS[b][:], Ln)
            tmp_t = st.tile([P, NCOLS], f32, tag=f"tmp{b}")
            nc.vector.tensor_tensor(tmp_t[:], term_t[:], lnS_t[:], sub)
            nc.vector.tensor_tensor(
                out_sb[:, b * NCOL:(b + 1) * NCOL], tmp_t[:], lnPS_t[:], add
            )
            nc.sync.dma_start(
                out_d[:, b * NCOL:(b + 1) * NCOL], out_sb[:, b * NCOL:(b + 1) * NCOL]
            )

    return nc


def _split_multi_waits(nc):
    """walrus's codegen allows a single embedded sync-wait per compute
    instruction; Tile sometimes emits two (e.g. ACT + DMA deps on one TT).
    Hoist all-but-one wait into standalone EventSemaphore instructions
    placed immediately before, on the same engine. Applied at BIR-JSON
    serialization time so CoreSim (which handles multi-wait fine) is
    untouched."""
    import json

    orig = nc.to_json_bytes

    def patched():
        bj = json.loads(orig())
        for fn in bj["functions"]:
            for blk in fn["blocks"]:
                new = []
                for inst in blk["instructions"]:
                    si = inst.get("sync_info") or {}
                    waits = si.get("on_wait") or []
                    if len(waits) > 1 and inst.get("opcode") != "EventSemaphore":
                        for i, w in enumerate(waits[:-1]):
                            new.append({
                                "debug": inst.get("debug"),
                                "engine": inst["engine"],
                                "ins": [],
                                "name": f"{inst['name']}-sw{i}",
                                "opcode": "EventSemaphore",
                                "outs": [],
                                "sync_info": {"on_update": [], "on_wait": [w]},
                            })
                        si["on_wait"] = [waits[-1]]
                    new.append(inst)
                blk["instructions"] = new
        return json.dumps(bj).encode()

    nc.to_json_bytes = patched
    return nc


def get_nc():
    global _CACHED_NC
    if _CACHED_NC is None:
        _CACHED_NC = _split_multi_waits(_build_nc())
    return _CACHED_NC


def make_in_maps(inputs):
    """Shard the full inputs into per-core in_maps for run_bass_kernel_spmd."""
    import ml_dtypes
    bf16 = ml_dtypes.bfloat16
    f32 = np.float32
    arr = {k: np.asarray(v) for k, v in inputs.items()}
    target = np.asarray(arr["target"]).astype(np.int64).reshape(B)
    onehot = np.zeros((B, C), dtype=f32)
    onehot[np.arange(B), target] = 1.0

    # bf16 full tensors once (0.5*log_std folded in on the host)
    big = {}
    for bn, srcs in (("bt", ("log_std_t", "eps_prior_t", "eps_t", "mean_t")),
                     ("bs", ("log_std_s", "eps_prior_s", "eps_s", "mean_s"))):
        mats = []
        for i, s in enumerate(srcs):
            a = np.asarray(arr[s], dtype=f32)
            if i == 0:
                a = a * 0.5
            mats.append(a.astype(bf16))
        big[bn] = mats

    in_maps = []
    for cidx in range(NCORES):
        sl = slice(cidx * RPC, (cidx + 1) * RPC)
        m = {}
        for bn in BRANCHES:
            # [P, NCH, 4, R]: slice order [0.5*ls | prior | eps | mean],
            # each [D=128 partitions, R rows]
            pk = np.stack(
                [
                    np.ascontiguousarray(a[sl].T).reshape(P, NCH, R)
                    for a in big[bn]
                ],
                axis=2,
            )
            m[bn] = np.ascontiguousarray(pk.reshape(P, NCH, 4 * R))
        yoh = np.empty((P, 2 * YF), dtype=bf16)
        yoh[:, :YF] = np.ascontiguousarray(arr["y_zt"][sl], dtype=f32).reshape(P, YF).astype(bf16)
        yoh[:, YF:] = np.ascontiguousarray(onehot[sl]).reshape(P, YF).astype(bf16)
        m["yoh"] = yoh
        m["sz"] = np.ascontiguousarray(arr["s_zt"][sl], dtype=f32).reshape(P, SF).astype(bf16)
        in_maps.append(m)
    return in_maps


def combine(outs, current_step):
    """Host-side unshard: f64 reduce of per-row partials -> final f32 scalar."""
    tot = np.zeros(4, dtype=np.float64)
    for o in outs:
        o = o.reshape(P, 4, NCOL)
        tot += o.sum(axis=(0, 2), dtype=np.float64)
    L_zt, L_zs, L_t, Loss_e = tot / B
    frac = float(current_step) / STEP_SIZE
    lam_e = LAMBDA_E * GAMMA_E ** frac
    lam_od = LAMBDA_OD * GAMMA_OD ** frac
    val = L_t + lam_e * Loss_e + lam_od * (L_zt + L_zs)
    return np.array(val, dtype=np.float32)


def _install_ntff_hook():
    """Best-effort: register the axon NTFF profiling hook that the agent
    image's antenv package is missing, so trace=True yields exec_time_ns."""
    try:
        import sys, types
        import antenv
        if "antenv.axon_hooks" in sys.modules:
            return True
        sys.path.insert(0, "/root/.axon_site/trn_agent_boot")
        import trn_boot
        mod = types.ModuleType("antenv.axon_hooks")
        _h = {}
        mod.set_axon_ntff_profile_hook = lambda h: _h.__setitem__("h", h)
        mod.get_axon_ntff_profile_hook = lambda: _h.get("h")
        sys.modules["antenv.axon_hooks"] = mod
        antenv.axon_hooks = mod
        mod.set_axon_ntff_profile_hook(
            trn_boot._ntff_profile_via_ctypes("/opt/axon/libaxon_pjrt.so")
        )
        import concourse.bass_utils as bu
        bu.upload_artifacts = lambda tmpdir: str(tmpdir)
        return True
    except Exception:
        return False


def kernel(**inputs):
    global LAST_EXEC_NS
    from concourse.bass_utils import run_bass_kernel_spmd

    trace = os.environ.get("BASS_KERNEL_TRACE", "0") == "1"
    if trace:
        trace = _install_ntff_hook()

    nc = get_nc()
    in_maps = make_in_maps(inputs)
    res = run_bass_kernel_spmd(
        nc, in_maps, list(range(NCORES)), trace=trace
    )
    LAST_EXEC_NS = res.exec_time_ns
    outs = [r["out"] for r in res.results]
    cs = inputs.get("current_step", 500)
    return combine(outs, int(np.asarray(cs)))
